# revision 5
# baseline (speedup 1.0000x reference)
"""AttentionBlock (GroupNorm + QKV 1x1conv + full attention + proj + residual)
for Trainium2, data-parallel over (batch, query-half) across 8 NeuronCores.

fp8 redesign: all large matmuls run as float8e4 DoubleRow (contraction 256 in
one instruction at 0.5 cyc/row = 4x the fp32r rate).

  - Host pre-casts x to fp8 (x8) and sends weights in bf16 (one packed DMA).
  - GroupNorm's per-channel affine (a, d from stats) is folded into the QKV
    weights on device: W' = fp8(W_bf16 * a[c_in]); bias' = W'@fp8(d/a) + b.
    x8 then feeds the QKV DoubleRow matmuls directly - no affine pass over
    the 4096-pixel activations.
  - Scores S^T[keys, q] via DR (contract 256 channels); exp on ScalarE with
    a -4 shift (cancels in softmax; keeps e4m3 below its 240 max) writing
    fp8; PV + softmax-denominator via DR over paired key blocks.
  - The whole attention is one flat software pipeline over 64 slots
    (4 query chunks x 16 key-block pairs): scores for slot g and PV for
    slot g-PVLAG are emitted together, so the PE never waits on ScalarE's
    exp, and each chunk's tail (1/D broadcast, normalize, proj, residual,
    store) is interleaved into the next chunk's first slots. V is produced
    streaming inside chunk 0; Q for chunk c+1 during chunk c. The final
    chunk uses a fast tail: bf16 proj of the unnormalized PV runs in
    parallel with the 1/D chain (a per-query scale commutes through the
    channel mix), with the residual+bias term precomputed mid-stream.
  - ScalarE (exp, ~67us busy) is the bottleneck engine: everything else is
    kept on PE/DVE or in the head phase. TimelineSim: 106.5us/core vs
    233.1us for the fp32r baseline; HW absmax rel err 9.7e-3 (gate 2e-2).

Per-core plan (core c: batch b=c//2, query-half h=c%2): host rolls x[b]'s
pixel axis so this core's 2048 queries are columns 0:2048 (attention is
permutation-invariant over keys; GroupNorm stats are permutation-invariant,
so a single SPMD program serves all cores).

The toolchain's walrus build accepts only one sync-wait per instruction, so
a post-pass splits multi-wait instructions into NoOp chains (HW only; CoreSim
runs with split=False).
"""

import sys

if "/opt/trn_rl_repo" not in sys.path:
    sys.path.insert(0, "/opt/trn_rl_repo")

import numpy as np

import concourse.bass as bass
import concourse.mybir as mybir
import concourse.tile as tile
from concourse.bass_utils import run_bass_kernel_spmd

F32 = mybir.dt.float32
F32R = mybir.dt.float32r
F8 = mybir.dt.float8e4
BF16 = mybir.dt.bfloat16
AF = mybir.ActivationFunctionType
ALU = mybir.AluOpType
DR = mybir.MatmulPerfMode.DoubleRow

B, C, HH, WW = 4, 256, 64, 64
N = HH * WW          # 4096 pixels
G = 8                # groups
QH = N // 2          # queries per core
NCORES = 8
EPS = 1e-5
INV_CNT = 1.0 / (32 * N)   # 1 / elements per group
SM_SCALE = 1.0 / 16.0      # 1/sqrt(C)
ESHIFT = -4.0              # exp shift; cancels in softmax ratio

CT = C // 128        # 2 channel tiles
MB = N // 128        # 32 key blocks
UP = MB // 2         # 16 key-block pairs
NCH = QH // 512      # 4 query chunks per core
KCH = N // 512       # 8 pixel chunks
NSLOT = NCH * UP     # 64 pipeline slots
PVLAG = 4            # pv(g - PVLAG) emitted at slot g


# ---------------------------------------------------------------------------
# walrus in this env allows only ONE sync-wait command per instruction.
_ws_counter = [0]


def _split_block(b):
    new = []
    changed = False
    for ins in b.instructions:
        si = ins.sync_info
        if si is not None and si.on_wait and len(si.on_wait) > 1:
            waits = list(si.on_wait)
            for w in waits[:-1]:
                _ws_counter[0] += 1
                new.append(mybir.InstNoOp(
                    name=f"I-waitsplit-{_ws_counter[0]}",
                    engine=ins.engine,
                    sync_info=mybir.SyncInfo(on_wait=[w], on_update=[]),
                ))
            ins.sync_info = mybir.SyncInfo(
                on_wait=[waits[-1]], on_update=list(si.on_update or []))
            changed = True
        new.append(ins)
    if changed:
        b.instructions[:] = new
    for sub in getattr(b, "blocks", []) or []:
        _split_block(sub)


def split_multi_waits(nc):
    for b in nc.main_func.blocks:
        _split_block(b)
    return nc


# ---------------------------------------------------------------------------
def build(split=True):
    """split=True applies the walrus single-wait post-pass (required for HW;
    CoreSim's race-replay machinery chokes on the NoOp chains, so sim tests
    pass split=False)."""
    nc = bass.Bass()

    X8 = nc.dram_tensor("x8", [128, CT, N], F8, kind="ExternalInput")
    XRES = nc.dram_tensor("xres", [128, CT, QH], F32, kind="ExternalInput")
    WPACK = nc.dram_tensor("wpack", [128, CT, 4 * C], BF16,
                           kind="ExternalInput")
    SPACK = nc.dram_tensor("spack", [128, 12], F32, kind="ExternalInput")
    GMASK = nc.dram_tensor("gmask", [128, 4], F32R, kind="ExternalInput")
    GB5 = nc.dram_tensor("gb5", [4, 128], F32R, kind="ExternalInput")
    ONESROW = nc.dram_tensor("ones1x128", [1, 128], F32R, kind="ExternalInput")
    ONES8 = nc.dram_tensor("ones8", [128, 2, 128], F8, kind="ExternalInput")
    OUT = nc.dram_tensor("out", [C, QH], F32, kind="ExternalOutput")

    with tile.TileContext(nc) as tc, nc.allow_low_precision(
            reason="fp8 attention; validated ~1e-2 absmax rel vs fp64"):
        with tc.tile_pool(name="big", bufs=1) as big, \
             tc.tile_pool(name="small", bufs=1) as small, \
             tc.tile_pool(name="expp", bufs=5) as expp, \
             tc.tile_pool(name="attp", bufs=2) as attp, \
             tc.tile_pool(name="dbp", bufs=2) as dbp, \
             tc.tile_pool(name="outp", bufs=2) as outp, \
             tc.tile_pool(name="ps_s", bufs=2, space="PSUM") as ps_s, \
             tc.tile_pool(name="ps_acc", bufs=1, space="PSUM") as ps_acc, \
             tc.tile_pool(name="ps_m", bufs=1, space="PSUM") as ps_m:

            # ---------------- loads
            # The cost model serializes all transfers through one DMA-engine
            # pool: order matters. Small/critical tensors first, then x8 in
            # halves (stats pipeline on each half), bulky xres (first needed
            # ~35us in) last.
            NHALF = N // 2
            x8 = big.tile([128, CT, N], F8, tag="x8")
            for h in range(2):
                hs = slice(h * NHALF, (h + 1) * NHALF)
                nc.sync.dma_start(x8[:, :, hs], X8[:, :, hs])
            gmask = small.tile([128, 4], F32R, tag="gmask")
            nc.sync.dma_start(gmask[:], GMASK[:])
            gb5 = small.tile([4, 128], F32R, tag="gb5")
            nc.sync.dma_start(gb5[:], GB5[:])
            sp = small.tile([128, 12], F32, tag="sp")
            nc.sync.dma_start(sp[:], SPACK[:])
            wpk = small.tile([128, CT, 4 * C], BF16, tag="wpk")
            nc.sync.dma_start(wpk[:], WPACK[:])
            ones8 = small.tile([128, 2, 128], F8, tag="ones8")
            nc.sync.dma_start(ones8[:], ONES8[:])
            onesrow_t = small.tile([1, 128], F32R, tag="onesrow")
            nc.sync.dma_start(onesrow_t[:], ONESROW[:])
            xres = big.tile([128, CT, QH], F32, tag="xres")
            nc.sync.dma_start(xres[:], XRES[:])

            bq2, bk2, pb2 = sp[:, 0:2], sp[:, 2:4], sp[:, 4:6]
            gamma, beta = sp[:, 6:8], sp[:, 8:10]
            epsc = sp[:, 10:11]
            gbcast = gb5[0:4, :]
            onesrow = onesrow_t[:]
            wqb = wpk[:, :, 0 * C:1 * C]
            wkb = wpk[:, :, 1 * C:2 * C]
            wvb = wpk[:, :, 2 * C:3 * C]
            wpb = wpk[:, :, 3 * C:4 * C]

            nb4 = small.tile([128, 1], F32, tag="nb4")
            nc.vector.memset(nb4[:], ESHIFT)

            # ---------------- GroupNorm stats, pipelined with the x8 halves:
            # ScalarE squares (accum_out); sums split DVE (tile 0) / GpSimd
            # (tile 1, tensor_scalar_add with accum_out); partials per half
            scr = big.tile([128, NHALF], F32, tag="scr")
            psum = small.tile([128, 4], F32R, tag="psum")
            psq = small.tile([128, 4], F32R, tag="psq")
            for h in range(2):
                hs = slice(h * NHALF, (h + 1) * NHALF)
                for t in range(CT):
                    p = 2 * h + t
                    nc.vector.tensor_reduce(
                        psum[:, p:p + 1], x8[:, t, hs],
                        axis=mybir.AxisListType.X, op=ALU.add)
                    nc.scalar.activation(scr[:], x8[:, t, hs], AF.Square,
                                         accum_out=psq[:, p:p + 1])
            stats = [small.tile([128, 2], F32R, tag=f"st{t}", name=f"st{t}")
                     for t in range(CT)]
            for t in range(CT):
                nc.vector.tensor_add(stats[t][:, 0:1], psum[:, t:t + 1],
                                     psum[:, t + 2:t + 3])
                nc.vector.tensor_add(stats[t][:, 1:2], psq[:, t:t + 1],
                                     psq[:, t + 2:t + 3])


            a2 = small.tile([128, CT], F32, tag="a2")
            ai2 = small.tile([128, CT], F32, tag="ai2")
            dia2 = small.tile([128, CT], F32, tag="dia2")
            for t in range(CT):
                ps_g = ps_m.tile([4, 2], F32, tag="m")
                nc.tensor.matmul(ps_g[:], gmask[:], stats[t][:],
                                 start=True, stop=True)
                gstats = small.tile([4, 2], F32R, tag=f"gst{t}",
                                    name=f"gst{t}")
                nc.vector.tensor_copy(gstats[:], ps_g[:])
                ps_bc = ps_m.tile([128, 2], F32, tag="m")
                nc.tensor.matmul(ps_bc[:], gbcast, gstats[:],
                                 start=True, stop=True)
                mean = small.tile([128, 1], F32, tag=f"mean{t}",
                                  name=f"mean{t}")
                ex2 = small.tile([128, 1], F32, tag=f"ex2{t}", name=f"ex2{t}")
                nc.vector.tensor_scalar_mul(mean[:], ps_bc[:, 0:1], INV_CNT)
                nc.vector.tensor_scalar_mul(ex2[:], ps_bc[:, 1:2], INV_CNT)
                varn = small.tile([128, 1], F32, tag=f"varn{t}",
                                  name=f"varn{t}")
                nc.vector.scalar_tensor_tensor(
                    varn[:], mean[:], mean[:], ex2[:],
                    op0=ALU.mult, op1=ALU.subtract)
                lnv = small.tile([128, 1], F32, tag=f"lnv{t}", name=f"lnv{t}")
                nc.scalar.activation(lnv[:], varn[:], AF.Ln,
                                     bias=epsc, scale=-1.0)
                rstd = small.tile([128, 1], F32, tag=f"rstd{t}",
                                  name=f"rstd{t}")
                nc.scalar.activation(rstd[:], lnv[:], AF.Exp,
                                     bias=0.0, scale=-0.5)
                nc.vector.tensor_mul(a2[:, t:t + 1], rstd[:], gamma[:, t:t + 1])
                nc.vector.reciprocal(ai2[:, t:t + 1], a2[:, t:t + 1])
                # dia = d/a = beta/a - mean  in one pass
                nc.vector.scalar_tensor_tensor(
                    dia2[:, t:t + 1], beta[:, t:t + 1], ai2[:, t:t + 1],
                    mean[:], op0=ALU.mult, op1=ALU.subtract)

            # ---------------- weight prep
            wq8 = small.tile([128, CT, C], F8, tag="wq8")
            wk8 = small.tile([128, CT, C], F8, tag="wk8")
            wv8 = small.tile([128, CT, C], F8, tag="wv8")
            wp8 = small.tile([128, CT, C], F8, tag="wp8")
            for t in range(CT):
                nc.vector.tensor_scalar_mul(wk8[:, t, :], wkb[:, t, :],
                                            a2[:, t:t + 1])
                nc.vector.tensor_scalar_mul(wq8[:, t, :], wqb[:, t, :],
                                            a2[:, t:t + 1])
                nc.vector.tensor_scalar_mul(wv8[:, t, :], wvb[:, t, :],
                                            a2[:, t:t + 1])
                nc.scalar.copy(wp8[:, t, :], wpb[:, t, :])

            d8 = small.tile([128, CT, 1], F8, tag="d8")
            nc.vector.tensor_copy(d8[:, :, 0], dia2[:])

            bqd = small.tile([128, CT], F32, tag="bqd")
            bkd = small.tile([128, CT], F32, tag="bkd")
            bvd = small.tile([128, CT], F32, tag="bvd")
            for w8_, host_b, dst in ((wk8, bk2, bkd), (wq8, bq2, bqd),
                                     (wv8, None, bvd)):
                for ot in range(CT):
                    ps_b = ps_m.tile([128, 1], F32, tag="m")
                    nc.tensor.matmul(ps_b[:],
                                     w8_[:, :, ot * 128:(ot + 1) * 128],
                                     d8[:], start=True, stop=True,
                                     perf_mode=DR)
                    if host_b is not None:
                        nc.vector.tensor_add(dst[:, ot:ot + 1], ps_b[:],
                                             host_b[:, ot:ot + 1])
                    else:
                        nc.vector.tensor_copy(dst[:, ot:ot + 1], ps_b[:])
            bv8 = small.tile([128, CT, 1], F8, tag="bv8")
            nc.vector.tensor_copy(bv8[:, :, 0], bvd[:])
            pbt = small.tile([128, CT], F32, tag="pbt")
            for ot in range(CT):
                ps_b = ps_m.tile([128, 1], F32, tag="m")
                nc.tensor.matmul(ps_b[:], wp8[:, :, ot * 128:(ot + 1) * 128],
                                 bv8[:], start=True, stop=True, perf_mode=DR)
                nc.vector.tensor_add(pbt[:, ot:ot + 1], ps_b[:],
                                     pb2[:, ot:ot + 1])

            # ---------------- Q for chunk 0 first (cheap; st(0) needs it)
            q8 = big.tile([128, CT, QH], F8, tag="q8")

            def emit_q(ch):
                qs = slice(ch * 512, (ch + 1) * 512)
                for ot in range(CT):
                    ps_q = ps_m.tile([128, 512], F32, tag="m")
                    nc.tensor.matmul(ps_q[:],
                                     wq8[:, :, ot * 128:(ot + 1) * 128],
                                     x8[:, :, qs], start=True, stop=True,
                                     perf_mode=DR)
                    nc.vector.tensor_scalar_add(q8[:, ot, qs], ps_q[:],
                                                bqd[:, ot:ot + 1])

            emit_q(0)

            # ---------------- K full (bias fold + fp8; copies on ACT/DVE)
            k8 = big.tile([128, CT, N], F8, tag="k8")
            for kch in range(KCH):
                cs = slice(kch * 512, (kch + 1) * 512)
                ps_k = ps_s.tile([128, 2, 512], F32, tag="s")
                for ot in range(CT):
                    nc.tensor.matmul(ps_k[:, ot, :],
                                     wk8[:, :, ot * 128:(ot + 1) * 128],
                                     x8[:, :, cs], start=True, stop=True,
                                     perf_mode=DR)
                for ot in range(CT):
                    if (kch + ot) % 2 == 0:
                        nc.scalar.activation(k8[:, ot, cs], ps_k[:, ot, :],
                                             AF.Identity,
                                             bias=bkd[:, ot:ot + 1], scale=1.0)
                    else:
                        nc.vector.tensor_scalar_add(k8[:, ot, cs],
                                                    ps_k[:, ot, :],
                                                    bkd[:, ot:ot + 1])

            # ---------------- attention: flat 64-slot pipeline
            vt8 = big.tile([128, MB, C], F8, tag="vt8")

            def emit_v(up):
                ps_v = ps_m.tile([128, 2, 256], F32, tag="m")
                for j in range(2):
                    u = 2 * up + j
                    nc.tensor.matmul(ps_v[:, j, :],
                                     x8[:, :, u * 128:(u + 1) * 128],
                                     wv8[:], start=True, stop=True,
                                     perf_mode=DR)
                nc.vector.tensor_copy(vt8[:, 2 * up:2 * up + 2, :], ps_v[:])

            # per-chunk state for the flat pipeline
            es = [None] * NSLOT
            ps_att = {}
            ps_d = {}

            def emit_st(g):
                ch, up = divmod(g, UP)
                qs = slice(ch * 512, (ch + 1) * 512)
                ps_st = ps_s.tile([128, 2, 512], F32, tag="s")
                for j in range(2):
                    u = 2 * up + j
                    nc.tensor.matmul(ps_st[:, j, :],
                                     k8[:, :, u * 128:(u + 1) * 128],
                                     q8[:, :, qs], start=True, stop=True,
                                     perf_mode=DR)
                e = expp.tile([128, 2, 512], F8, tag="e", name=f"e{g}")
                nc.scalar.activation(e[:], ps_st[:], AF.Exp,
                                     bias=nb4[:], scale=SM_SCALE)
                es[g] = e

            def emit_pvd(g):
                ch, up = divmod(g, UP)
                nc.tensor.matmul(ps_d[ch][:], ones8[:], es[g][:],
                                 start=(up == 0), stop=(up == UP - 1),
                                 perf_mode=DR)

            def emit_pvatt(g):
                ch, up = divmod(g, UP)
                for ct in range(CT):
                    nc.tensor.matmul(
                        ps_att[ch][ct][:],
                        vt8[:, 2 * up:2 * up + 2, ct * 128:(ct + 1) * 128],
                        es[g][:], start=(up == 0), stop=(up == UP - 1),
                        perf_mode=DR)
                es[g] = None

            # chunk tail, split into three stages interleaved with the next
            # chunk's first slots
            def tail_a(ch):
                # denominator reciprocal + partition broadcast
                drec = dbp.tile([1, 512], F32R, tag="drec", name=f"drec{ch}")
                nc.vector.reciprocal(drec[:], ps_d[ch][0:1, :])
                ps_db = ps_m.tile([128, 512], F32, tag="m")
                nc.tensor.matmul(ps_db[:], onesrow, drec[:],
                                 start=True, stop=True)
                db = dbp.tile([128, 512], F32, tag="db", name=f"db{ch}")
                nc.vector.tensor_copy(db[:], ps_db[:])
                return db

            def tail_m(ch, db):
                # normalize into fp8 (frees ps_att for the next chunk)
                att = attp.tile([128, CT, 512], F8, tag="att",
                                name=f"att{ch}")
                for ct in range(CT):
                    nc.vector.tensor_mul(att[:, ct, :], ps_att[ch][ct][:],
                                         db[:])
                return att

            def tail_c(ch, att):
                qs = slice(ch * 512, (ch + 1) * 512)
                for ot in range(CT):
                    ps_p = ps_s.tile([128, 2, 512], F32, tag="s")
                    nc.tensor.matmul(ps_p[:, 0, :],
                                     wp8[:, :, ot * 128:(ot + 1) * 128],
                                     att[:], start=True, stop=True,
                                     perf_mode=DR)
                    o_t = outp.tile([128, 512], F32, tag="o", name=f"o{ch}{ot}")
                    nc.vector.scalar_tensor_tensor(
                        o_t[:], ps_p[:, 0, :], pbt[:, ot:ot + 1],
                        xres[:, ot, qs], op0=ALU.add, op1=ALU.add)
                    nc.sync.dma_start(OUT[ot * 128:(ot + 1) * 128, qs],
                                      o_t[:])

            att_pending = {}

            def emit_tail_am(ch):
                att_pending[ch] = tail_m(ch, tail_a(ch))

            # residual + folded bias for the final chunk, precomputed
            # mid-stream off the critical path (see g == 32 below)
            rfin = outp.tile([128, CT, 512], F32, tag="rfin", bufs=1)
            lqs = slice((NCH - 1) * 512, NCH * 512)

            # pv emission schedule: lag PVLAG in steady state, catching up
            # over the last two st slots so only pv(NSLOT-1) trails the
            # final exp
            next_pv = [0]

            def pv_target(g):
                if g < NSLOT - 2:
                    return g - PVLAG
                if g == NSLOT - 2:
                    return g - 3
                if g == NSLOT - 1:
                    return g - 1
                return NSLOT - 1

            def emit_pv_upto(tgt):
                while next_pv[0] <= tgt:
                    pg = next_pv[0]
                    pch, pup = divmod(pg, UP)
                    if pup == 0 and pch > 0:
                        # previous chunk's D and PV are complete; run its
                        # reciprocal chain + normalize before this chunk's
                        # first accumulating matmuls reuse the PSUM banks
                        emit_tail_am(pch - 1)
                    emit_pvd(pg)
                    emit_pvatt(pg)
                    if pup == 1 and pch > 0:
                        tail_c(pch - 1, att_pending.pop(pch - 1))
                    next_pv[0] += 1

            for g in range(NSLOT):
                ch, up = divmod(g, UP)
                if up == 0:
                    ps_att[ch] = [
                        ps_acc.tile([128, 512], F32, tag=f"att{ct}",
                                    name=f"psatt{ch}{ct}")
                        for ct in range(CT)]
                    ps_d[ch] = ps_acc.tile([128, 512], F32, tag="psd",
                                           name=f"psd{ch}")
                emit_st(g)
                # stream V a few pairs ahead during chunk 0
                if ch == 0:
                    if up == 0:
                        for k_ in (0, 1, 2, 3):
                            emit_v(k_)
                    elif up <= 12:
                        emit_v(up + 3)
                if ch < NCH - 1 and up == 8:
                    emit_q(ch + 1)
                if g == 2 * UP:
                    for ot in range(CT):
                        nc.vector.tensor_scalar_add(rfin[:, ot, :],
                                                    xres[:, ot, lqs],
                                                    pbt[:, ot:ot + 1])
                emit_pv_upto(pv_target(g))
            emit_pv_upto(NSLOT - 1)

            # ---- final chunk fast tail: 1/D chain and bf16 proj of the
            # unnormalized PV run in parallel; normalization happens after
            # proj (1/D is a per-query scale; it commutes through the
            # channel mix)
            lch = NCH - 1
            drec = dbp.tile([1, 512], F32R, tag="drec", name="drecF")
            nc.vector.reciprocal(drec[:], ps_d[lch][0:1, :])
            ps_db = ps_m.tile([128, 512], F32, tag="m")
            nc.tensor.matmul(ps_db[:], onesrow, drec[:],
                             start=True, stop=True)
            attb = attp.tile([128, CT, 512], BF16, tag="attb")
            for ct in range(CT):
                nc.vector.tensor_copy(attb[:, ct, :], ps_att[lch][ct][:])
            db = dbp.tile([128, 512], F32, tag="db", name="dbF")
            nc.vector.tensor_copy(db[:], ps_db[:])
            for ot in range(CT):
                ps_p = ps_s.tile([128, 2, 512], F32, tag="s")
                for ct in range(CT):
                    nc.tensor.matmul(
                        ps_p[:, 0, :],
                        wpb[:, ct, ot * 128:(ot + 1) * 128],
                        attb[:, ct, :], start=(ct == 0),
                        stop=(ct == CT - 1))
                t1 = outp.tile([128, 512], F32, tag="t1", name=f"t1{ot}")
                nc.vector.tensor_mul(t1[:], ps_p[:, 0, :], db[:])
                o_t = outp.tile([128, 512], F32, tag="o", name=f"oF{ot}")
                nc.vector.tensor_add(o_t[:], t1[:], rfin[:, ot, :])
                if ot == 0:
                    nc.scalar.dma_start(OUT[ot * 128:(ot + 1) * 128, lqs],
                                        o_t[:])
                else:
                    nc.sync.dma_start(OUT[ot * 128:(ot + 1) * 128, lqs],
                                      o_t[:])

    if split:
        split_multi_waits(nc)
    return nc


_NC_CACHE = None


def _get_nc():
    global _NC_CACHE
    if _NC_CACHE is None:
        _NC_CACHE = build()
    return _NC_CACHE


def make_in_maps(x, gamma, beta, w_qkv, b_qkv, w_proj, b_proj):
    import ml_dtypes
    f8t = np.dtype(ml_dtypes.float8_e4m3)
    bft = np.dtype(ml_dtypes.bfloat16)

    x = np.asarray(x, np.float32).reshape(B, C, N)
    gamma = np.asarray(gamma, np.float32)
    beta = np.asarray(beta, np.float32)
    w_qkv = np.asarray(w_qkv, np.float32)
    b_qkv = np.asarray(b_qkv, np.float32)
    w_proj = np.asarray(w_proj, np.float32)
    b_proj = np.asarray(b_proj, np.float32)

    wq, wk, wv = w_qkv[0:C], w_qkv[C:2 * C], w_qkv[2 * C:3 * C]
    bq, bk, bv = b_qkv[0:C], b_qkv[C:2 * C], b_qkv[2 * C:3 * C]
    pb2 = (w_proj @ bv + b_proj).astype(np.float32)

    def col2(v):
        return v.reshape(CT, 128).T.astype(np.float32)

    def wtile(w):
        # [128(p), CT(c_in tile), C(c_out)]; w is [c_out, c_in]
        return w.T.reshape(CT, 128, C).transpose(1, 0, 2)

    wpack = np.concatenate(
        [wtile(w) for w in (wq, wk, wv, w_proj)], axis=2).astype(bft)
    spack = np.zeros((128, 12), np.float32)
    spack[:, 0:2] = col2(bq)
    spack[:, 2:4] = col2(bk)
    spack[:, 4:6] = col2(pb2)
    spack[:, 6:8] = col2(gamma)
    spack[:, 8:10] = col2(beta)
    spack[:, 10] = EPS

    gmask = np.zeros((128, 4), np.float32)
    for p in range(128):
        gmask[p, p // 32] = 1.0
    gb5 = np.ascontiguousarray(gmask.T)
    ones8h = np.zeros((128, 2, 128), f8t)
    ones8h[:, :, 0] = np.ones((128, 2), np.float32).astype(f8t)

    common = {
        "wpack": np.ascontiguousarray(wpack),
        "spack": spack,
        "gmask": gmask,
        "gb5": gb5,
        "ones1x128": np.ones((1, 128), np.float32),
        "ones8": ones8h,
    }

    in_maps = []
    for core in range(NCORES):
        b, half = core // 2, core % 2
        qoff = half * QH
        xc = np.concatenate([x[b][:, qoff:], x[b][:, :qoff]], axis=1)
        m = dict(common)
        m["x8"] = np.ascontiguousarray(
            xc.reshape(CT, 128, N).transpose(1, 0, 2)).astype(f8t)
        m["xres"] = np.ascontiguousarray(
            xc[:, :QH].reshape(CT, 128, QH).transpose(1, 0, 2))
        in_maps.append(m)
    return in_maps


def gather_out(results):
    out = np.empty((B, C, N), np.float32)
    for core in range(NCORES):
        b, half = core // 2, core % 2
        qoff = half * QH
        out[b][:, qoff:qoff + QH] = results[core]["out"]
    return out.reshape(B, C, HH, WW)


def kernel(x, gamma, beta, w_qkv, b_qkv, w_proj, b_proj, **run_kwargs):
    nc = _get_nc()
    in_maps = make_in_maps(x, gamma, beta, w_qkv, b_qkv, w_proj, b_proj)
    res = run_bass_kernel_spmd(nc, in_maps, core_ids=list(range(NCORES)),
                               **run_kwargs)
    out = gather_out(res.results)
    kernel.last_results = res
    return out


# revision 11
# speedup vs baseline: 1.0062x; 1.0062x over previous
"""AttentionBlock (GroupNorm + QKV 1x1conv + full attention + proj + residual)
for Trainium2, data-parallel over (batch, query-half) across 8 NeuronCores.

fp8 redesign: all large matmuls run as float8e4 DoubleRow (contraction 256 in
one instruction at 0.5 cyc/row = 4x the fp32r rate).

  - Host pre-casts x to fp8 (x8) and sends weights in bf16 (one packed DMA).
  - GroupNorm's per-channel affine (a, d from stats) is folded into the QKV
    weights on device: W' = fp8(W_bf16 * a[c_in]); bias' = W'@fp8(d/a) + b.
    x8 then feeds the QKV DoubleRow matmuls directly - no affine pass over
    the 4096-pixel activations.
  - Scores S^T[keys, q] via DR (contract 256 channels); exp on ScalarE with
    a -4 shift (cancels in softmax; keeps e4m3 below its 240 max) writing
    fp8; PV + softmax-denominator via DR over paired key blocks.
  - The whole attention is one flat software pipeline over 64 slots
    (4 query chunks x 16 key-block pairs): scores for slot g and PV for
    slot g-PVLAG are emitted together, so the PE never waits on ScalarE's
    exp, and each chunk's tail (1/D broadcast, normalize, proj, residual,
    store) is interleaved into the next chunk's first slots. V is produced
    streaming inside chunk 0; Q for chunk c+1 during chunk c. The final
    chunk uses a fast tail: bf16 proj of the unnormalized PV runs in
    parallel with the 1/D chain (a per-query scale commutes through the
    channel mix), with the residual+bias term precomputed mid-stream.
  - ScalarE (exp, ~67us busy) is the bottleneck engine: everything else is
    kept on PE/DVE or in the head phase. TimelineSim: 106.5us/core vs
    233.1us for the fp32r baseline; HW absmax rel err 9.7e-3 (gate 2e-2).

Per-core plan (core c: batch b=c//2, query-half h=c%2): host rolls x[b]'s
pixel axis so this core's 2048 queries are columns 0:2048 (attention is
permutation-invariant over keys; GroupNorm stats are permutation-invariant,
so a single SPMD program serves all cores).

The toolchain's walrus build accepts only one sync-wait per instruction, so
a post-pass splits multi-wait instructions into NoOp chains (HW only; CoreSim
runs with split=False).
"""

import sys

if "/opt/trn_rl_repo" not in sys.path:
    sys.path.insert(0, "/opt/trn_rl_repo")

import numpy as np

import concourse.bass as bass
import concourse.mybir as mybir
import concourse.tile as tile
from concourse.bass_utils import run_bass_kernel_spmd

F32 = mybir.dt.float32
F32R = mybir.dt.float32r
F8 = mybir.dt.float8e4
BF16 = mybir.dt.bfloat16
AF = mybir.ActivationFunctionType
ALU = mybir.AluOpType
DR = mybir.MatmulPerfMode.DoubleRow

B, C, HH, WW = 4, 256, 64, 64
N = HH * WW          # 4096 pixels
G = 8                # groups
QH = N // 2          # queries per core
NCORES = 8
EPS = 1e-5
INV_CNT = 1.0 / (32 * N)   # 1 / elements per group
SM_SCALE = 1.0 / 16.0      # 1/sqrt(C)
ESHIFT = -4.0              # exp shift; cancels in softmax ratio

CT = C // 128        # 2 channel tiles
MB = N // 128        # 32 key blocks
UP = MB // 2         # 16 key-block pairs
NCH = QH // 512      # 4 query chunks per core
KCH = N // 512       # 8 pixel chunks
NSLOT = NCH * UP     # 64 pipeline slots
PVLAG = 4            # pv(g - PVLAG) emitted at slot g


# ---------------------------------------------------------------------------
# walrus in this env allows only ONE sync-wait command per instruction.
_ws_counter = [0]


def _split_block(b):
    new = []
    changed = False
    for ins in b.instructions:
        si = ins.sync_info
        if si is not None and si.on_wait and len(si.on_wait) > 1:
            waits = list(si.on_wait)
            for w in waits[:-1]:
                _ws_counter[0] += 1
                new.append(mybir.InstNoOp(
                    name=f"I-waitsplit-{_ws_counter[0]}",
                    engine=ins.engine,
                    sync_info=mybir.SyncInfo(on_wait=[w], on_update=[]),
                ))
            ins.sync_info = mybir.SyncInfo(
                on_wait=[waits[-1]], on_update=list(si.on_update or []))
            changed = True
        new.append(ins)
    if changed:
        b.instructions[:] = new
    for sub in getattr(b, "blocks", []) or []:
        _split_block(sub)


def split_multi_waits(nc):
    for b in nc.main_func.blocks:
        _split_block(b)
    return nc


# ---------------------------------------------------------------------------
def build(split=True):
    """split=True applies the walrus single-wait post-pass (required for HW;
    CoreSim's race-replay machinery chokes on the NoOp chains, so sim tests
    pass split=False)."""
    nc = bass.Bass()

    X8 = nc.dram_tensor("x8", [128, CT, N], F8, kind="ExternalInput")
    XRES = nc.dram_tensor("xres", [128, CT, QH], F32, kind="ExternalInput")
    WPACK = nc.dram_tensor("wpack", [128, CT, 4 * C], BF16,
                           kind="ExternalInput")
    SPACK = nc.dram_tensor("spack", [128, 12], F32, kind="ExternalInput")
    GMASK = nc.dram_tensor("gmask", [128, 4], F32R, kind="ExternalInput")
    GB5 = nc.dram_tensor("gb5", [4, 128], F32R, kind="ExternalInput")
    ONESROW = nc.dram_tensor("ones1x128", [1, 128], F32R, kind="ExternalInput")
    ONES8 = nc.dram_tensor("ones8", [128, 2, 128], F8, kind="ExternalInput")
    OUT = nc.dram_tensor("out", [C, QH], F32, kind="ExternalOutput")

    with tile.TileContext(nc) as tc, nc.allow_low_precision(
            reason="fp8 attention; validated ~1e-2 absmax rel vs fp64"):
        with tc.tile_pool(name="big", bufs=1) as big, \
             tc.tile_pool(name="small", bufs=1) as small, \
             tc.tile_pool(name="expp", bufs=5) as expp, \
             tc.tile_pool(name="attp", bufs=2) as attp, \
             tc.tile_pool(name="dbp", bufs=2) as dbp, \
             tc.tile_pool(name="outp", bufs=2) as outp, \
             tc.tile_pool(name="ps_s", bufs=2, space="PSUM") as ps_s, \
             tc.tile_pool(name="ps_acc", bufs=1, space="PSUM") as ps_acc, \
             tc.tile_pool(name="ps_m", bufs=1, space="PSUM") as ps_m:

            # ---------------- loads
            # The cost model serializes all transfers through one DMA-engine
            # pool: order matters. Small/critical tensors first, then x8 in
            # halves (stats pipeline on each half), bulky xres (first needed
            # ~35us in) last.
            NHALF = N // 2
            x8 = big.tile([128, CT, N], F8, tag="x8")
            for h in range(2):
                hs = slice(h * NHALF, (h + 1) * NHALF)
                nc.sync.dma_start(x8[:, :, hs], X8[:, :, hs])
            gmask = small.tile([128, 4], F32R, tag="gmask")
            nc.sync.dma_start(gmask[:], GMASK[:])
            gb5 = small.tile([4, 128], F32R, tag="gb5")
            nc.sync.dma_start(gb5[:], GB5[:])
            sp = small.tile([128, 12], F32, tag="sp")
            nc.sync.dma_start(sp[:], SPACK[:])
            wpk = small.tile([128, CT, 4 * C], BF16, tag="wpk")
            nc.sync.dma_start(wpk[:], WPACK[:])
            ones8 = small.tile([128, 2, 128], F8, tag="ones8")
            nc.sync.dma_start(ones8[:], ONES8[:])
            onesrow_t = small.tile([1, 128], F32R, tag="onesrow")
            nc.sync.dma_start(onesrow_t[:], ONESROW[:])
            xres = big.tile([128, CT, QH], F32, tag="xres")
            nc.sync.dma_start(xres[:], XRES[:])

            bq2, bk2, pb2 = sp[:, 0:2], sp[:, 2:4], sp[:, 4:6]
            gamma, beta = sp[:, 6:8], sp[:, 8:10]
            epsc = sp[:, 10:11]
            gbcast = gb5[0:4, :]
            onesrow = onesrow_t[:]
            wqb = wpk[:, :, 0 * C:1 * C]
            wkb = wpk[:, :, 1 * C:2 * C]
            wvb = wpk[:, :, 2 * C:3 * C]
            wpb = wpk[:, :, 3 * C:4 * C]

            nb4 = small.tile([128, 1], F32, tag="nb4")
            nc.vector.memset(nb4[:], ESHIFT)

            # ---------------- GroupNorm stats, pipelined with the x8 halves:
            # ScalarE squares (accum_out); sums split DVE (tile 0) / GpSimd
            # (tile 1, tensor_scalar_add with accum_out); partials per half
            scr = big.tile([128, NHALF], F32, tag="scr")
            psum = small.tile([128, 4], F32R, tag="psum")
            psq = small.tile([128, 4], F32R, tag="psq")
            for h in range(2):
                hs = slice(h * NHALF, (h + 1) * NHALF)
                for t in range(CT):
                    p = 2 * h + t
                    nc.vector.tensor_reduce(
                        psum[:, p:p + 1], x8[:, t, hs],
                        axis=mybir.AxisListType.X, op=ALU.add)
                    nc.scalar.activation(scr[:], x8[:, t, hs], AF.Square,
                                         accum_out=psq[:, p:p + 1])
            stats = [small.tile([128, 2], F32R, tag=f"st{t}", name=f"st{t}")
                     for t in range(CT)]
            for t in range(CT):
                nc.vector.tensor_add(stats[t][:, 0:1], psum[:, t:t + 1],
                                     psum[:, t + 2:t + 3])
                nc.vector.tensor_add(stats[t][:, 1:2], psq[:, t:t + 1],
                                     psq[:, t + 2:t + 3])


            a2 = small.tile([128, CT], F32, tag="a2")
            ai2 = small.tile([128, CT], F32, tag="ai2")
            dia2 = small.tile([128, CT], F32, tag="dia2")
            for t in range(CT):
                ps_g = ps_m.tile([4, 2], F32, tag="m")
                nc.tensor.matmul(ps_g[:], gmask[:], stats[t][:],
                                 start=True, stop=True)
                gstats = small.tile([4, 2], F32R, tag=f"gst{t}",
                                    name=f"gst{t}")
                nc.vector.tensor_copy(gstats[:], ps_g[:])
                ps_bc = ps_m.tile([128, 2], F32, tag="m")
                nc.tensor.matmul(ps_bc[:], gbcast, gstats[:],
                                 start=True, stop=True)
                mean = small.tile([128, 1], F32, tag=f"mean{t}",
                                  name=f"mean{t}")
                ex2 = small.tile([128, 1], F32, tag=f"ex2{t}", name=f"ex2{t}")
                nc.vector.tensor_scalar_mul(mean[:], ps_bc[:, 0:1], INV_CNT)
                nc.vector.tensor_scalar_mul(ex2[:], ps_bc[:, 1:2], INV_CNT)
                varn = small.tile([128, 1], F32, tag=f"varn{t}",
                                  name=f"varn{t}")
                nc.vector.scalar_tensor_tensor(
                    varn[:], mean[:], mean[:], ex2[:],
                    op0=ALU.mult, op1=ALU.subtract)
                lnv = small.tile([128, 1], F32, tag=f"lnv{t}", name=f"lnv{t}")
                nc.scalar.activation(lnv[:], varn[:], AF.Ln,
                                     bias=epsc, scale=-1.0)
                rstd = small.tile([128, 1], F32, tag=f"rstd{t}",
                                  name=f"rstd{t}")
                nc.scalar.activation(rstd[:], lnv[:], AF.Exp,
                                     bias=0.0, scale=-0.5)
                nc.vector.tensor_mul(a2[:, t:t + 1], rstd[:], gamma[:, t:t + 1])
                nc.vector.reciprocal(ai2[:, t:t + 1], a2[:, t:t + 1])
                # dia = d/a = beta/a - mean  in one pass
                nc.vector.scalar_tensor_tensor(
                    dia2[:, t:t + 1], beta[:, t:t + 1], ai2[:, t:t + 1],
                    mean[:], op0=ALU.mult, op1=ALU.subtract)

            # ---------------- weight prep
            wq8 = small.tile([128, CT, C], F8, tag="wq8")
            wk8 = small.tile([128, CT, C], F8, tag="wk8")
            wv8 = small.tile([128, CT, C], F8, tag="wv8")
            wp8 = small.tile([128, CT, C], F8, tag="wp8")
            for t in range(CT):
                nc.vector.tensor_scalar_mul(wk8[:, t, :], wkb[:, t, :],
                                            a2[:, t:t + 1])
                nc.vector.tensor_scalar_mul(wq8[:, t, :], wqb[:, t, :],
                                            a2[:, t:t + 1])
                nc.vector.tensor_scalar_mul(wv8[:, t, :], wvb[:, t, :],
                                            a2[:, t:t + 1])
                nc.scalar.copy(wp8[:, t, :], wpb[:, t, :])

            d8 = small.tile([128, CT, 1], F8, tag="d8")
            nc.vector.tensor_copy(d8[:, :, 0], dia2[:])

            bqd = small.tile([128, CT], F32, tag="bqd")
            bkd = small.tile([128, CT], F32, tag="bkd")
            bvd = small.tile([128, CT], F32, tag="bvd")
            for w8_, host_b, dst in ((wk8, bk2, bkd), (wq8, bq2, bqd),
                                     (wv8, None, bvd)):
                for ot in range(CT):
                    ps_b = ps_m.tile([128, 1], F32, tag="m")
                    nc.tensor.matmul(ps_b[:],
                                     w8_[:, :, ot * 128:(ot + 1) * 128],
                                     d8[:], start=True, stop=True,
                                     perf_mode=DR)
                    if host_b is not None:
                        nc.vector.tensor_add(dst[:, ot:ot + 1], ps_b[:],
                                             host_b[:, ot:ot + 1])
                    else:
                        nc.vector.tensor_copy(dst[:, ot:ot + 1], ps_b[:])
            bv8 = small.tile([128, CT, 1], F8, tag="bv8")
            nc.vector.tensor_copy(bv8[:, :, 0], bvd[:])
            pbt = small.tile([128, CT], F32, tag="pbt")
            for ot in range(CT):
                ps_b = ps_m.tile([128, 1], F32, tag="m")
                nc.tensor.matmul(ps_b[:], wp8[:, :, ot * 128:(ot + 1) * 128],
                                 bv8[:], start=True, stop=True, perf_mode=DR)
                nc.vector.tensor_add(pbt[:, ot:ot + 1], ps_b[:],
                                     pb2[:, ot:ot + 1])

            # ---------------- Q for chunk 0 first (cheap; st(0) needs it)
            q8 = big.tile([128, CT, QH], F8, tag="q8")

            def emit_q(ch):
                qs = slice(ch * 512, (ch + 1) * 512)
                for ot in range(CT):
                    ps_q = ps_m.tile([128, 512], F32, tag="m")
                    nc.tensor.matmul(ps_q[:],
                                     wq8[:, :, ot * 128:(ot + 1) * 128],
                                     x8[:, :, qs], start=True, stop=True,
                                     perf_mode=DR)
                    nc.vector.tensor_scalar_add(q8[:, ot, qs], ps_q[:],
                                                bqd[:, ot:ot + 1])

            emit_q(0)

            # ---------------- K full (bias fold + fp8; copies on ACT/DVE)
            k8 = big.tile([128, CT, N], F8, tag="k8")
            for kch in range(KCH):
                cs = slice(kch * 512, (kch + 1) * 512)
                ps_k = ps_s.tile([128, 2, 512], F32, tag="s")
                for ot in range(CT):
                    nc.tensor.matmul(ps_k[:, ot, :],
                                     wk8[:, :, ot * 128:(ot + 1) * 128],
                                     x8[:, :, cs], start=True, stop=True,
                                     perf_mode=DR)
                for ot in range(CT):
                    if (kch + ot) % 2 == 0:
                        nc.scalar.activation(k8[:, ot, cs], ps_k[:, ot, :],
                                             AF.Identity,
                                             bias=bkd[:, ot:ot + 1], scale=1.0)
                    else:
                        nc.vector.tensor_scalar_add(k8[:, ot, cs],
                                                    ps_k[:, ot, :],
                                                    bkd[:, ot:ot + 1])

            # ---------------- attention: flat 64-slot pipeline
            vt8 = big.tile([128, MB, C], F8, tag="vt8")

            def emit_v(up):
                ps_v = ps_m.tile([128, 2, 256], F32, tag="m")
                for j in range(2):
                    u = 2 * up + j
                    nc.tensor.matmul(ps_v[:, j, :],
                                     x8[:, :, u * 128:(u + 1) * 128],
                                     wv8[:], start=True, stop=True,
                                     perf_mode=DR)
                nc.vector.tensor_copy(vt8[:, 2 * up:2 * up + 2, :], ps_v[:])

            # per-chunk state for the flat pipeline
            es = [None] * NSLOT
            ps_att = {}
            ps_d = {}

            def emit_st(g):
                ch, up = divmod(g, UP)
                qs = slice(ch * 512, (ch + 1) * 512)
                ps_st = ps_s.tile([128, 2, 512], F32, tag="s")
                for j in range(2):
                    u = 2 * up + j
                    nc.tensor.matmul(ps_st[:, j, :],
                                     k8[:, :, u * 128:(u + 1) * 128],
                                     q8[:, :, qs], start=True, stop=True,
                                     perf_mode=DR)
                e = expp.tile([128, 2, 512], F8, tag="e", name=f"e{g}")
                nc.scalar.activation(e[:], ps_st[:], AF.Exp,
                                     bias=nb4[:], scale=SM_SCALE)
                es[g] = e

            def emit_pvd(g):
                ch, up = divmod(g, UP)
                nc.tensor.matmul(ps_d[ch][:], ones8[:], es[g][:],
                                 start=(up == 0), stop=(up == UP - 1),
                                 perf_mode=DR)

            def emit_pvatt(g):
                ch, up = divmod(g, UP)
                for ct in range(CT):
                    nc.tensor.matmul(
                        ps_att[ch][ct][:],
                        vt8[:, 2 * up:2 * up + 2, ct * 128:(ct + 1) * 128],
                        es[g][:], start=(up == 0), stop=(up == UP - 1),
                        perf_mode=DR)
                es[g] = None

            # chunk tail, split into three stages interleaved with the next
            # chunk's first slots
            def tail_a(ch):
                # denominator reciprocal + partition broadcast
                drec = dbp.tile([1, 512], F32R, tag="drec", name=f"drec{ch}")
                nc.vector.reciprocal(drec[:], ps_d[ch][0:1, :])
                ps_db = ps_m.tile([128, 512], F32, tag="m")
                nc.tensor.matmul(ps_db[:], onesrow, drec[:],
                                 start=True, stop=True)
                db = dbp.tile([128, 512], F32, tag="db", name=f"db{ch}")
                nc.vector.tensor_copy(db[:], ps_db[:])
                return db

            def tail_m(ch, db):
                # normalize into fp8 (frees ps_att for the next chunk)
                att = attp.tile([128, CT, 512], F8, tag="att",
                                name=f"att{ch}")
                for ct in range(CT):
                    nc.vector.tensor_mul(att[:, ct, :], ps_att[ch][ct][:],
                                         db[:])
                return att

            def tail_c(ch, att):
                qs = slice(ch * 512, (ch + 1) * 512)
                for ot in range(CT):
                    ps_p = ps_s.tile([128, 2, 512], F32, tag="s")
                    nc.tensor.matmul(ps_p[:, 0, :],
                                     wp8[:, :, ot * 128:(ot + 1) * 128],
                                     att[:], start=True, stop=True,
                                     perf_mode=DR)
                    o_t = outp.tile([128, 512], F32, tag="o", name=f"o{ch}{ot}")
                    nc.vector.scalar_tensor_tensor(
                        o_t[:], ps_p[:, 0, :], pbt[:, ot:ot + 1],
                        xres[:, ot, qs], op0=ALU.add, op1=ALU.add)
                    nc.sync.dma_start(OUT[ot * 128:(ot + 1) * 128, qs],
                                      o_t[:])

            att_pending = {}

            def emit_tail_am(ch):
                att_pending[ch] = tail_m(ch, tail_a(ch))

            # residual + folded bias for the final chunk, precomputed
            # mid-stream off the critical path (see g == 32 below)
            rfin = outp.tile([128, CT, 512], F32, tag="rfin", bufs=1)
            lqs = slice((NCH - 1) * 512, NCH * 512)

            # pv emission schedule: lag PVLAG in steady state, catching up
            # over the last two st slots so only pv(NSLOT-1) trails the
            # final exp
            next_pv = [0]

            def pv_target(g):
                if g < NSLOT - 2:
                    return g - PVLAG
                if g == NSLOT - 2:
                    return g - 3
                if g == NSLOT - 1:
                    return g - 1
                return NSLOT - 1

            def emit_pv_upto(tgt):
                while next_pv[0] <= tgt:
                    pg = next_pv[0]
                    pch, pup = divmod(pg, UP)
                    if pup == 0 and pch > 0:
                        # previous chunk's D and PV are complete; run its
                        # reciprocal chain + normalize before this chunk's
                        # first accumulating matmuls reuse the PSUM banks
                        emit_tail_am(pch - 1)
                    emit_pvd(pg)
                    emit_pvatt(pg)
                    if pup == 1 and pch > 0:
                        tail_c(pch - 1, att_pending.pop(pch - 1))
                    next_pv[0] += 1

            for g in range(NSLOT):
                ch, up = divmod(g, UP)
                if up == 0:
                    ps_att[ch] = [
                        ps_acc.tile([128, 512], F32, tag=f"att{ct}",
                                    name=f"psatt{ch}{ct}")
                        for ct in range(CT)]
                    ps_d[ch] = ps_acc.tile([128, 512], F32, tag="psd",
                                           name=f"psd{ch}")
                emit_st(g)
                # stream V a few pairs ahead during chunk 0
                if ch == 0:
                    if up == 0:
                        for k_ in (0, 1, 2, 3):
                            emit_v(k_)
                    elif up <= 12:
                        emit_v(up + 3)
                if ch < NCH - 1 and up == 8:
                    emit_q(ch + 1)
                if g == 2 * UP + 6:
                    for ot in range(CT):
                        nc.vector.tensor_scalar_add(rfin[:, ot, :],
                                                    xres[:, ot, lqs],
                                                    pbt[:, ot:ot + 1])
                emit_pv_upto(pv_target(g))
            emit_pv_upto(NSLOT - 1)

            # ---- final chunk fast tail: 1/D chain and bf16 proj of the
            # unnormalized PV run in parallel; normalization happens after
            # proj (1/D is a per-query scale; it commutes through the
            # channel mix)
            lch = NCH - 1
            drec = dbp.tile([1, 512], F32R, tag="drec", name="drecF")
            nc.vector.reciprocal(drec[:], ps_d[lch][0:1, :])
            ps_db = ps_m.tile([128, 512], F32, tag="m")
            nc.tensor.matmul(ps_db[:], onesrow, drec[:],
                             start=True, stop=True)
            attb = attp.tile([128, CT, 512], BF16, tag="attb")
            # ScalarE is idle after the last exp: give it one of the two
            # PV casts so the final DVE chain shortens
            nc.scalar.copy(attb[:, 0, :], ps_att[lch][0][:])
            nc.vector.tensor_copy(attb[:, 1, :], ps_att[lch][1][:])
            db = dbp.tile([128, 512], F32, tag="db", name="dbF")
            nc.vector.tensor_copy(db[:], ps_db[:])
            for ot in range(CT):
                ps_p = ps_s.tile([128, 2, 512], F32, tag="s")
                for ct in range(CT):
                    nc.tensor.matmul(
                        ps_p[:, 0, :],
                        wpb[:, ct, ot * 128:(ot + 1) * 128],
                        attb[:, ct, :], start=(ct == 0),
                        stop=(ct == CT - 1))
                t1 = outp.tile([128, 512], F32, tag="t1", name=f"t1{ot}")
                nc.vector.tensor_mul(t1[:], ps_p[:, 0, :], db[:])
                o_t = outp.tile([128, 512], F32, tag="o", name=f"oF{ot}")
                nc.vector.tensor_add(o_t[:], t1[:], rfin[:, ot, :])
                if ot == 0:
                    nc.scalar.dma_start(OUT[ot * 128:(ot + 1) * 128, lqs],
                                        o_t[:])
                else:
                    nc.sync.dma_start(OUT[ot * 128:(ot + 1) * 128, lqs],
                                      o_t[:])

    if split:
        split_multi_waits(nc)
    return nc


_NC_CACHE = None


def _get_nc():
    global _NC_CACHE
    if _NC_CACHE is None:
        _NC_CACHE = build()
    return _NC_CACHE


def make_in_maps(x, gamma, beta, w_qkv, b_qkv, w_proj, b_proj):
    import ml_dtypes
    f8t = np.dtype(ml_dtypes.float8_e4m3)
    bft = np.dtype(ml_dtypes.bfloat16)

    x = np.asarray(x, np.float32).reshape(B, C, N)
    gamma = np.asarray(gamma, np.float32)
    beta = np.asarray(beta, np.float32)
    w_qkv = np.asarray(w_qkv, np.float32)
    b_qkv = np.asarray(b_qkv, np.float32)
    w_proj = np.asarray(w_proj, np.float32)
    b_proj = np.asarray(b_proj, np.float32)

    wq, wk, wv = w_qkv[0:C], w_qkv[C:2 * C], w_qkv[2 * C:3 * C]
    bq, bk, bv = b_qkv[0:C], b_qkv[C:2 * C], b_qkv[2 * C:3 * C]
    pb2 = (w_proj @ bv + b_proj).astype(np.float32)

    def col2(v):
        return v.reshape(CT, 128).T.astype(np.float32)

    def wtile(w):
        # [128(p), CT(c_in tile), C(c_out)]; w is [c_out, c_in]
        return w.T.reshape(CT, 128, C).transpose(1, 0, 2)

    wpack = np.concatenate(
        [wtile(w) for w in (wq, wk, wv, w_proj)], axis=2).astype(bft)
    spack = np.zeros((128, 12), np.float32)
    spack[:, 0:2] = col2(bq)
    spack[:, 2:4] = col2(bk)
    spack[:, 4:6] = col2(pb2)
    spack[:, 6:8] = col2(gamma)
    spack[:, 8:10] = col2(beta)
    spack[:, 10] = EPS

    gmask = np.zeros((128, 4), np.float32)
    for p in range(128):
        gmask[p, p // 32] = 1.0
    gb5 = np.ascontiguousarray(gmask.T)
    ones8h = np.zeros((128, 2, 128), f8t)
    ones8h[:, :, 0] = np.ones((128, 2), np.float32).astype(f8t)

    common = {
        "wpack": np.ascontiguousarray(wpack),
        "spack": spack,
        "gmask": gmask,
        "gb5": gb5,
        "ones1x128": np.ones((1, 128), np.float32),
        "ones8": ones8h,
    }

    in_maps = []
    for core in range(NCORES):
        b, half = core // 2, core % 2
        qoff = half * QH
        xc = np.concatenate([x[b][:, qoff:], x[b][:, :qoff]], axis=1)
        m = dict(common)
        m["x8"] = np.ascontiguousarray(
            xc.reshape(CT, 128, N).transpose(1, 0, 2)).astype(f8t)
        m["xres"] = np.ascontiguousarray(
            xc[:, :QH].reshape(CT, 128, QH).transpose(1, 0, 2))
        in_maps.append(m)
    return in_maps


def gather_out(results):
    out = np.empty((B, C, N), np.float32)
    for core in range(NCORES):
        b, half = core // 2, core % 2
        qoff = half * QH
        out[b][:, qoff:qoff + QH] = results[core]["out"]
    return out.reshape(B, C, HH, WW)


def kernel(x, gamma, beta, w_qkv, b_qkv, w_proj, b_proj, **run_kwargs):
    nc = _get_nc()
    in_maps = make_in_maps(x, gamma, beta, w_qkv, b_qkv, w_proj, b_proj)
    res = run_bass_kernel_spmd(nc, in_maps, core_ids=list(range(NCORES)),
                               **run_kwargs)
    out = gather_out(res.results)
    kernel.last_results = res
    return out
